# revision 35
# baseline (speedup 1.0000x reference)
"""Trainium2 Bass kernel for the contrastive loss problem.

Math (per batch element b, one NeuronCore each):
  feat (C=64, N=4000), prob (N,);  normal = prob < 0.5
  featn = l2-normalize(feat, axis=C);  s = (featn.T @ featn) / 0.1
  pos_loss = -log(mean_{m!=n, both normal} exp(s_mn) + 1e-6)
  neg_loss = mean_{m normal, n anomaly} -log(1 - sigmoid(s_mn) + 1e-6)
  result   = sum_b valid_b * (pos+neg) / max(#valid, 1)

Strategy: data-parallel over batch (8 batches -> 8 cores). Host sorts points
normal-first, scales by sqrt(10) (so the Gram matrix is directly s), and
builds two zero-padded (64, 2048) bf16 operands:
  rp = first min(nn, 2048) normalized normal points
  rn = normalized anomaly points (na <= 2048 for the target inputs)
The device computes, per 128-row block j of rp:
  pos: exp-sum of rp_blk.T @ rp[:, 128j:2048]   (block upper triangle incl.
       the diagonal block; ScalarE fused accumulate out of PSUM)
  neg: softplus-sum of rp_blk.T @ rn, via Exp -> DVE product-fold (8 deep,
       bf16) -> Ln+accumulate (softplus(s) = ln(1+e^s); the |error| vs the
       reference's -log(1-sigmoid(s)+eps) is ~eps*(1+e^s), negligible).
Masked (zero-padded) entries contribute exp(0)=1 / factor 2.0 exactly; the
host subtracts them in closed form. The diagonal 128x128 blocks and any
normal points beyond 2048 are handled on the host in f64 (cheap), which is
what lets the device stream stay at 16 blocks.
"""

import numpy as np

RW = 2048          # padded region width = 16 blocks of 128
NBLK = RW // 128   # 16 row blocks
UNIT = 2048        # PSUM staging tile width (4 banks); ping-pong 2 tiles
N_CORES = 8
EPS = 1e-6
_SQ10 = float(np.sqrt(10.0))


def _make_stream(block_col_ranges):
    """Cut a concatenated (block, colrange) matmul output stream into <=512
    segments that never cross a 512-stream boundary (PSUM bank safety).
    Returns (segments, total): segments = (block, c0, c1, stream_pos)."""
    segs, pos = [], 0
    for j, cs, ce in block_col_ranges:
        c = cs
        while c < ce:
            take = min(512 - (pos % 512), ce - c)
            segs.append((j, c, c + take, pos))
            pos += take
            c += take
    return segs, pos


# pos U-stream: per block j, cols [128(j+1), 2048) — strict upper blocks
# only; the diagonal 128x128 blocks are recomputed on the host (D_full).
_POS_SEGS, _POS_LEN = _make_stream(
    [(j, 128 * (j + 1), RW) for j in range(NBLK - 1)])
_NU_P = (_POS_LEN + UNIT - 1) // UNIT   # 8 units (last 1024 wide)
_NU_N = NBLK                            # 16 neg units of exactly 2048
_N_GRP = (_NU_N + 15) // 16             # Ln groups (16 units -> 2048 cols)

_compiled = None


def _build():
    import concourse.bass as bass
    import concourse.mybir as mybir
    import concourse.tile as tile
    from concourse import bacc
    from concourse.hw_specs import get_activation_tables

    # Exp and Ln both live in the 'natural_log_exp_and_others' table set, but
    # the default placement resolves them to different sets, causing a ~1.3us
    # ACT table reload on every Exp<->Ln alternation. Steer the placement to
    # the shared set by hiding Exp/Ln from every other set. Set ORDER must be
    # preserved: act_func_set_id is the index into act_info.json's sets, and
    # NRT loads table content by that index.
    def _tables_pref(arch):
        t = get_activation_tables(arch)
        pref = "natural_log_exp_and_others"
        AFt = mybir.ActivationFunctionType
        return {k: (v if k == pref else v - {AFt.Exp, AFt.Ln})
                for k, v in t.items()}

    bacc.get_activation_tables = _tables_pref

    f32 = mybir.dt.float32
    bf16 = mybir.dt.bfloat16
    AF = mybir.ActivationFunctionType

    nc = bacc.Bacc("TRN2", target_bir_lowering=False, debug=False,
                   num_devices=N_CORES)
    rp_d = nc.dram_tensor("rp", [64, RW], bf16, kind="ExternalInput")
    rn_d = nc.dram_tensor("rn", [64, RW], bf16, kind="ExternalInput")
    accp_d = nc.dram_tensor("accp", [128, 2 * _NU_P], f32,
                            kind="ExternalOutput")
    accn_d = nc.dram_tensor("accn", [128, _N_GRP], f32, kind="ExternalOutput")

    with tile.TileContext(nc) as tc:
        with (
            tc.tile_pool(name="sb", bufs=1) as sb,
            tc.tile_pool(name="scratch", bufs=2) as scratch_pool,
            tc.tile_pool(name="psum", bufs=4, space=bass.MemorySpace.PSUM) as pp,
        ):
            rp_sb = sb.tile([64, RW], bf16, tag="rp")
            rn_sb = sb.tile([64, RW], bf16, tag="rn")
            # chunked loads on two queues: the first neg unit's matmul
            # segments only need rn[:, c:c+512] and rp[:, 0:128], so the PE
            # can start ~1us earlier than with monolithic loads.
            for c in range(0, RW, 512):
                nc.sync.dma_start(out=rn_sb[:, c:c + 512],
                                  in_=rn_d.ap()[:, c:c + 512])
            nc.gpsimd.dma_start(out=rp_sb[:, 0:128], in_=rp_d.ap()[:, 0:128])
            nc.gpsimd.dma_start(out=rp_sb[:, 128:RW],
                                in_=rp_d.ap()[:, 128:RW])

            acc_p = sb.tile([128, 2 * _NU_P], f32, tag="accp")
            acc_n = sb.tile([128, _N_GRP], f32, tag="accn")
            # per-unit persistent fold buffers: the last two fold levels are
            # deferred into the pos phase (DVE is idle there), so each neg
            # unit keeps its gt slice alive instead of cycling a ring
            gt_all = sb.tile([128, _NU_N * (UNIT // 4)], bf16, tag="gt_all")
            kt_all = sb.tile([128, _NU_N * (UNIT // 8)], bf16, tag="kt_all")
            ltw_t = sb.tile([128, _NU_N * (UNIT // 16)], bf16, tag="ltw")

            # PSUM is 4 half-tiles of 1024 so the PE always has a free tile
            # to start on (avoids idle -> LOW p-state restarts).
            def emit_matmuls_range(ptile, segs, lo, hi, rhs_sb):
                for (j, c0, c1, pos) in segs:
                    if lo <= pos < hi:
                        nc.tensor.matmul(
                            ptile[:, pos - lo:pos - lo + (c1 - c0)],
                            rp_sb[:, j * 128:(j + 1) * 128],
                            rhs_sb[:, c0:c1],
                            start=True, stop=True,
                        )

            # The fold tail (kt/lt levels) and the single Ln are deferred
            # into the pos phase, where the DVE and ACT have idle slots.
            state = {"pending": None}

            def flush_pending_ln():
                if state["pending"] is not None:
                    ltw, g, w = state["pending"]
                    ld = scratch_pool.tile([128, UNIT], bf16, tag="fold_o",
                                           name="ld")
                    nc.scalar.activation(ld[:, :w], ltw[:, :w], AF.Ln,
                                         accum_out=acc_n[:, g:g + 1])
                    state["pending"] = None

            # neg phase first: sum softplus(s) = sum ln(1+e^s), with groups
            # of 8 (1+e^s) factors folded by the DVE in bf16 so the Ln pass
            # is 8x narrower (max product (1+e^10)^8 ~ 5.7e34 is inside bf16
            # range). ln(prod) decomposes exactly for masked columns because
            # their factor is exactly 2.0 in bf16.
            for u in range(_NU_N):
                w = UNIT
                h1, h2, h3, h4 = w // 2, w // 4, w // 8, w // 16
                et = scratch_pool.tile([128, UNIT], bf16, tag="scratch")
                for half in range(2):
                    ptile = pp.tile([128, UNIT // 2], f32, tag="unit")
                    for c in range(0, UNIT // 2, 512):
                        nc.tensor.matmul(
                            ptile[:, c:c + 512],
                            rp_sb[:, u * 128:(u + 1) * 128],
                            rn_sb[:, half * h1 + c:half * h1 + c + 512],
                            start=True, stop=True,
                        )
                    nc.scalar.activation(et[:, half * h1:half * h1 + h1],
                                         ptile[:], AF.Exp)
                # factors are (1+e^s)/4 so a 16-deep fold stays inside bf16
                # range (max (1+e^5.9)^16/4^16 ~ e^72); masked factors become
                # exactly 0.5 (exponent shift, no rounding).
                ft = scratch_pool.tile([128, UNIT], bf16, tag="fold_f")
                nc.vector.tensor_scalar(
                    out=ft[:, :w], in0=et[:, :w], scalar1=0.25, scalar2=0.25,
                    op0=mybir.AluOpType.mult, op1=mybir.AluOpType.add)
                ht = scratch_pool.tile([128, UNIT // 2], bf16, tag="fold_h")
                nc.vector.tensor_tensor(
                    ht[:, :h1], ft[:, :h1], ft[:, h1:w],
                    op=mybir.AluOpType.mult)
                nc.vector.tensor_tensor(
                    gt_all[:, u * h2:u * h2 + h2], ht[:, :h2], ht[:, h2:h1],
                    op=mybir.AluOpType.mult)

            # pos phase (exp-sum, fused accumulate straight out of PSUM);
            # the neg fold tail + Ln are emitted inside it so they overlap
            # pos ACT/PE work on the otherwise-idle DVE
            for u in range(_NU_P):
                base = u * UNIT
                w = min(UNIT, _POS_LEN - base)
                st = scratch_pool.tile([128, UNIT], bf16, tag="scratch")
                for half in range(2):
                    lo = base + half * (UNIT // 2)
                    wh = min(UNIT // 2, _POS_LEN - lo)
                    if wh <= 0:
                        continue
                    ptile = pp.tile([128, UNIT // 2], f32, tag="unit")
                    emit_matmuls_range(ptile, _POS_SEGS, lo, lo + wh, rp_sb)
                    off = half * (UNIT // 2)
                    nc.scalar.activation(
                        st[:, off:off + wh], ptile[:, :wh], AF.Exp,
                        accum_out=acc_p[:, 2 * u + half:2 * u + half + 1])
                if u == 0:
                    h2, h3, h4 = UNIT // 4, UNIT // 8, UNIT // 16
                    for v in range(_NU_N):
                        g0 = v * h2
                        nc.vector.tensor_tensor(
                            kt_all[:, v * h3:v * h3 + h3],
                            gt_all[:, g0:g0 + h3],
                            gt_all[:, g0 + h3:g0 + h2],
                            op=mybir.AluOpType.mult)
                        k0 = v * h3
                        nc.vector.tensor_tensor(
                            ltw_t[:, v * h4:v * h4 + h4],
                            kt_all[:, k0:k0 + h4],
                            kt_all[:, k0 + h4:k0 + h3],
                            op=mybir.AluOpType.mult)
                    state["pending"] = (ltw_t, 0, _NU_N * h4)
                flush_pending_ln()
            flush_pending_ln()

            # raw accumulators out; final reduction happens on host in f64
            nc.sync.dma_start(out=accp_d.ap(), in_=acc_p[:])
            nc.sync.dma_start(out=accn_d.ap(), in_=acc_n[:])

    nc.compile()
    return nc


def _get_compiled():
    global _compiled
    if _compiled is None:
        _compiled = _build()
    return _compiled


def _prepare(features, anomaly_prob):
    """Host prep: per batch -> (rp, rn) operands + metadata for combine."""
    import ml_dtypes
    feat_all = np.asarray(features, dtype=np.float32)[..., 0]      # (8,64,4000)
    prob_all = np.asarray(anomaly_prob, dtype=np.float32)[:, 0, :, 0]
    BS, C, N = feat_all.shape
    in_maps, metas = [], []
    for b in range(BS):
        feat, prob = feat_all[b], prob_all[b]
        normal = prob < np.float32(0.5)
        nn = int(normal.sum())
        na = N - nn
        # device covers 2048 normal points and 2048 anomaly columns; the
        # host cleans up a small normal overflow. Bail out to numpy if the
        # input distribution is far from the expected ~50/50 split.
        if na > RW or nn - RW > 512:
            return None, None
        norms = np.sqrt(np.sum(feat * feat, axis=0, dtype=np.float32))
        sc = (np.float32(_SQ10) /
              np.maximum(norms, np.float32(1e-12))).astype(np.float32)
        featsc = feat * sc[None, :]
        nd = min(nn, RW)
        fn_all = featsc[:, normal]            # (64, nn) normal points
        fa_all = featsc[:, ~normal]           # (64, na)
        rp = np.zeros((C, RW), np.float32)
        rp[:, :nd] = fn_all[:, :nd]
        rn = np.zeros((C, RW), np.float32)
        rn[:, :na] = fa_all
        rp16 = rp.astype(ml_dtypes.bfloat16)
        rn16 = rn.astype(ml_dtypes.bfloat16)
        # f64 views of the bf16-rounded operands (same values the PE sees)
        rp64 = rp16.astype(np.float64)
        rn64 = rn16.astype(np.float64)
        ov64 = fn_all[:, nd:nn].astype(ml_dtypes.bfloat16).astype(np.float64)

        # host-side diagonal-block sums (f64): D_full = sum over same-block
        # normal-normal pairs (incl. m=n), S2 = sum over m=n only.
        D_full = 0.0
        S2 = 0.0
        for j in range(NBLK):
            r0, r1 = 128 * j, min(128 * (j + 1), nd)
            if r0 >= r1:
                break
            blk = rp64[:, r0:r1]
            sblk = blk.T @ blk
            e = np.exp(sblk)
            D_full += float(e.sum())
            S2 += float(np.trace(e))

        # overflow normals (beyond RW): pos pairs vs all normals + selves,
        # neg pairs vs all anomalies, in f64.
        pos_extra = 0.0
        neg_extra = 0.0
        if nn > nd:
            dev64 = rp64[:, :nd]
            s_cross = ov64.T @ dev64              # (novf, nd)
            pos_extra += 2.0 * float(np.exp(s_cross).sum())
            s_oo = ov64.T @ ov64
            e_oo = np.exp(s_oo)
            pos_extra += float(e_oo.sum()) - float(np.trace(e_oo))
            s_on = ov64.T @ rn64[:, :na]
            sig = 1.0 / (1.0 + np.exp(-s_on))
            neg_extra += float(-np.log(1.0 - sig + EPS).sum())

        metas.append((nn, na, nd, D_full, S2, pos_extra, neg_extra))
        in_maps.append({"rp": rp16, "rn": rn16})
    return in_maps, metas


def _combine(results, metas):
    LN2 = float(np.log(np.float32(2.0)))
    per_batch, n_valid = [], 0
    for r, (nn, na, nd, D_full, S2, pos_extra, neg_extra) in zip(results, metas):
        TC = float(np.asarray(r["accp"], dtype=np.float64).sum())
        TN = float(np.asarray(r["accn"], dtype=np.float64).sum())
        # pos: U-stream block j covers rows [128j,128j+128) x cols
        # [128(j+1), RW). Real (non-padded) entries need row < nd, col < nd.
        fake_c = 0
        for j in range(NBLK - 1):
            rows = min(max(nd - 128 * j, 0), 128)
            cols = max(nd - 128 * (j + 1), 0)
            fake_c += 128 * (RW - 128 * (j + 1)) - rows * cols
        TU_real = TC - float(fake_c)          # exp(0) = 1 exactly
        pos_sum = 2.0 * TU_real + (D_full - S2) + pos_extra
        pos_loss = -np.log(pos_sum / max(nn * (nn - 1), 1) + EPS)
        # neg: stream is RW rows x RW cols; real entries (row < nd, col < na)
        # contribute softplus(s) - 2*ln2 each (the /4 factor scaling),
        # masked entries exactly -ln2 each.
        neg_sum = TN + LN2 * (RW * RW + nd * na) + neg_extra
        neg_loss = neg_sum / max(nn * na, 1)
        if nn >= 10 and na >= 5:
            n_valid += 1
            per_batch.append(pos_loss + neg_loss)
    total = np.sum(per_batch) / max(n_valid, 1) if per_batch else 0.0
    return np.asarray(total, dtype=np.float32)


def _numpy_fallback(features, anomaly_prob):
    feat_all = np.asarray(features, dtype=np.float32)[..., 0]
    prob_all = np.asarray(anomaly_prob, dtype=np.float32)[:, 0, :, 0]
    BS, C, N = feat_all.shape
    per_batch, n_valid = [], 0
    for b in range(BS):
        feat, prob = feat_all[b], prob_all[b]
        normal = prob < 0.5
        nn = int(normal.sum()); na = N - nn
        norms = np.sqrt(np.sum(feat * feat, axis=0, dtype=np.float32))
        fn = feat / np.maximum(norms, 1e-12)[None, :]
        s = (fn.T @ fn) / np.float32(0.1)
        nm, am = normal, ~normal
        eye = np.eye(N, dtype=bool)
        pm = nm[:, None] & nm[None, :] & ~eye
        pos_mean = np.where(pm, np.exp(s), 0.0).sum() / max(pm.sum(), 1)
        pos_loss = -np.log(pos_mean + EPS)
        cm = nm[:, None] & am[None, :]
        neg = np.where(cm, -np.log(1.0 - 1.0 / (1.0 + np.exp(-s)) + EPS),
                       0.0).sum() / max(cm.sum(), 1)
        if nn >= 10 and na >= 5:
            n_valid += 1
            per_batch.append(pos_loss + neg)
    total = np.sum(per_batch) / max(n_valid, 1) if per_batch else 0.0
    return np.asarray(total, dtype=np.float32)


def kernel(features, anomaly_prob):
    from concourse.bass_utils import run_bass_kernel_spmd
    in_maps, metas = _prepare(features, anomaly_prob)
    if in_maps is None:
        return _numpy_fallback(features, anomaly_prob)
    nc = _get_compiled()
    res = run_bass_kernel_spmd(nc, in_maps, list(range(N_CORES)))
    return _combine(res.results, metas)


# revision 36
# speedup vs baseline: 1.0412x; 1.0412x over previous
"""Trainium2 Bass kernel for the contrastive loss problem.

Math (per batch element b, one NeuronCore each):
  feat (C=64, N=4000), prob (N,);  normal = prob < 0.5
  featn = l2-normalize(feat, axis=C);  s = (featn.T @ featn) / 0.1
  pos_loss = -log(mean_{m!=n, both normal} exp(s_mn) + 1e-6)
  neg_loss = mean_{m normal, n anomaly} -log(1 - sigmoid(s_mn) + 1e-6)
  result   = sum_b valid_b * (pos+neg) / max(#valid, 1)

Strategy: data-parallel over batch (8 batches -> 8 cores). Host sorts points
normal-first, scales by sqrt(10) (so the Gram matrix is directly s), and
builds two zero-padded (64, 2048) bf16 operands:
  rp = first min(nn, 2048) normalized normal points
  rn = normalized anomaly points (na <= 2048 for the target inputs)
The device computes, per 128-row block j of rp:
  pos: exp-sum of rp_blk.T @ rp[:, 128j:2048]   (block upper triangle incl.
       the diagonal block; ScalarE fused accumulate out of PSUM)
  neg: softplus-sum of rp_blk.T @ rn, via Exp -> DVE product-fold (8 deep,
       bf16) -> Ln+accumulate (softplus(s) = ln(1+e^s); the |error| vs the
       reference's -log(1-sigmoid(s)+eps) is ~eps*(1+e^s), negligible).
Masked (zero-padded) entries contribute exp(0)=1 / factor 2.0 exactly; the
host subtracts them in closed form. The diagonal 128x128 blocks and any
normal points beyond 2048 are handled on the host in f64 (cheap), which is
what lets the device stream stay at 16 blocks.
"""

import numpy as np

RW = 2048          # padded region width = 16 blocks of 128
NBLK = RW // 128   # 16 row blocks
UNIT = 2048        # PSUM staging tile width (4 banks); ping-pong 2 tiles
N_CORES = 8
EPS = 1e-6
_SQ10 = float(np.sqrt(10.0))


def _make_stream(block_col_ranges):
    """Cut a concatenated (block, colrange) matmul output stream into <=512
    segments that never cross a 512-stream boundary (PSUM bank safety).
    Returns (segments, total): segments = (block, c0, c1, stream_pos)."""
    segs, pos = [], 0
    for j, cs, ce in block_col_ranges:
        c = cs
        while c < ce:
            take = min(512 - (pos % 512), ce - c)
            segs.append((j, c, c + take, pos))
            pos += take
            c += take
    return segs, pos


# pos U-stream: per block j, cols [128(j+1), 2048) — strict upper blocks
# only; the diagonal 128x128 blocks are recomputed on the host (D_full).
_POS_SEGS, _POS_LEN = _make_stream(
    [(j, 128 * (j + 1), RW) for j in range(NBLK - 1)])
_NU_P = (_POS_LEN + UNIT - 1) // UNIT   # 8 units (last 1024 wide)
_NU_N = NBLK                            # 16 neg units of exactly 2048
_N_GRP = (_NU_N + 15) // 16             # Ln groups (16 units -> 2048 cols)

_compiled = None


def _build():
    import concourse.bass as bass
    import concourse.mybir as mybir
    import concourse.tile as tile
    from concourse import bacc
    from concourse.hw_specs import get_activation_tables

    # Exp and Ln both live in the 'natural_log_exp_and_others' table set, but
    # the default placement resolves them to different sets, causing a ~1.3us
    # ACT table reload on every Exp<->Ln alternation. Steer the placement to
    # the shared set by hiding Exp/Ln from every other set. Set ORDER must be
    # preserved: act_func_set_id is the index into act_info.json's sets, and
    # NRT loads table content by that index.
    def _tables_pref(arch):
        t = get_activation_tables(arch)
        pref = "natural_log_exp_and_others"
        AFt = mybir.ActivationFunctionType
        return {k: (v if k == pref else v - {AFt.Exp, AFt.Ln})
                for k, v in t.items()}

    bacc.get_activation_tables = _tables_pref

    f32 = mybir.dt.float32
    bf16 = mybir.dt.bfloat16
    AF = mybir.ActivationFunctionType

    nc = bacc.Bacc("TRN2", target_bir_lowering=False, debug=False,
                   num_devices=N_CORES)
    rp_d = nc.dram_tensor("rp", [64, RW], bf16, kind="ExternalInput")
    rn_d = nc.dram_tensor("rn", [64, RW], bf16, kind="ExternalInput")
    accp_d = nc.dram_tensor("accp", [128, _NU_P], f32, kind="ExternalOutput")
    accn_d = nc.dram_tensor("accn", [128, _N_GRP], f32, kind="ExternalOutput")

    with tile.TileContext(nc) as tc:
        with (
            tc.tile_pool(name="sb", bufs=1) as sb,
            tc.tile_pool(name="scratch", bufs=2) as scratch_pool,
            tc.tile_pool(name="psum", bufs=2, space=bass.MemorySpace.PSUM) as pp,
        ):
            rp_sb = sb.tile([64, RW], bf16, tag="rp")
            rn_sb = sb.tile([64, RW], bf16, tag="rn")
            # chunked loads on two queues: the first neg unit's matmul
            # segments only need rn[:, c:c+512] and rp[:, 0:128], so the PE
            # can start ~1us earlier than with monolithic loads.
            for c in range(0, RW, 512):
                nc.sync.dma_start(out=rn_sb[:, c:c + 512],
                                  in_=rn_d.ap()[:, c:c + 512])
            nc.gpsimd.dma_start(out=rp_sb[:, 0:128], in_=rp_d.ap()[:, 0:128])
            nc.gpsimd.dma_start(out=rp_sb[:, 128:RW],
                                in_=rp_d.ap()[:, 128:RW])

            acc_p = sb.tile([128, _NU_P], f32, tag="accp")
            acc_n = sb.tile([128, _N_GRP], f32, tag="accn")
            # per-unit persistent fold buffers: the last two fold levels are
            # deferred into the pos phase (DVE is idle there), so each neg
            # unit keeps its gt slice alive instead of cycling a ring
            gt_all = sb.tile([128, _NU_N * (UNIT // 4)], bf16, tag="gt_all")
            kt_all = sb.tile([128, _NU_N * (UNIT // 8)], bf16, tag="kt_all")
            ltw_t = sb.tile([128, _NU_N * (UNIT // 16)], bf16, tag="ltw")

            def emit_matmuls(ptile, segs, total, u, rhs_sb):
                base = u * UNIT
                w = min(UNIT, total - base)
                for (j, c0, c1, pos) in segs:
                    if base <= pos < base + w:
                        nc.tensor.matmul(
                            ptile[:, pos - base:pos - base + (c1 - c0)],
                            rp_sb[:, j * 128:(j + 1) * 128],
                            rhs_sb[:, c0:c1],
                            start=True, stop=True,
                        )
                return w

            # The fold tail (kt/lt levels) and the single Ln are deferred
            # into the pos phase, where the DVE and ACT have idle slots.
            state = {"pending": None}

            def flush_pending_ln():
                if state["pending"] is not None:
                    ltw, g, w = state["pending"]
                    ld = scratch_pool.tile([128, UNIT], bf16, tag="fold_o",
                                           name="ld")
                    nc.scalar.activation(ld[:, :w], ltw[:, :w], AF.Ln,
                                         accum_out=acc_n[:, g:g + 1])
                    state["pending"] = None

            # neg phase first: sum softplus(s) = sum ln(1+e^s), with groups
            # of 8 (1+e^s) factors folded by the DVE in bf16 so the Ln pass
            # is 8x narrower (max product (1+e^10)^8 ~ 5.7e34 is inside bf16
            # range). ln(prod) decomposes exactly for masked columns because
            # their factor is exactly 2.0 in bf16.
            for u in range(_NU_N):
                ptile = pp.tile([128, UNIT], f32, tag="unit")
                for c in range(0, UNIT, 512):
                    nc.tensor.matmul(
                        ptile[:, c:c + 512],
                        rp_sb[:, u * 128:(u + 1) * 128],
                        rn_sb[:, c:c + 512],
                        start=True, stop=True,
                    )
                w = UNIT
                h1, h2, h3, h4 = w // 2, w // 4, w // 8, w // 16
                et = scratch_pool.tile([128, UNIT], bf16, tag="scratch")
                nc.scalar.activation(et[:, :w], ptile[:, :w], AF.Exp)
                flush_pending_ln()
                # factors are (1+e^s)/4 so a 16-deep fold stays inside bf16
                # range (max (1+e^5.9)^16/4^16 ~ e^72); masked factors become
                # exactly 0.5 (exponent shift, no rounding).
                ft = scratch_pool.tile([128, UNIT], bf16, tag="fold_f")
                nc.vector.tensor_scalar(
                    out=ft[:, :w], in0=et[:, :w], scalar1=0.25, scalar2=0.25,
                    op0=mybir.AluOpType.mult, op1=mybir.AluOpType.add)
                ht = scratch_pool.tile([128, UNIT // 2], bf16, tag="fold_h")
                nc.vector.tensor_tensor(
                    ht[:, :h1], ft[:, :h1], ft[:, h1:w],
                    op=mybir.AluOpType.mult)
                nc.vector.tensor_tensor(
                    gt_all[:, u * h2:u * h2 + h2], ht[:, :h2], ht[:, h2:h1],
                    op=mybir.AluOpType.mult)

            # pos phase (exp-sum, fused accumulate straight out of PSUM);
            # the neg fold tail + Ln are emitted inside it so they overlap
            # pos ACT/PE work on the otherwise-idle DVE
            for u in range(_NU_P):
                ptile = pp.tile([128, UNIT], f32, tag="unit")
                w = emit_matmuls(ptile, _POS_SEGS, _POS_LEN, u, rp_sb)
                st = scratch_pool.tile([128, UNIT], bf16, tag="scratch")
                nc.scalar.activation(st[:, :w], ptile[:, :w], AF.Exp,
                                     accum_out=acc_p[:, u:u + 1])
                if u == 0:
                    h2, h3, h4 = UNIT // 4, UNIT // 8, UNIT // 16
                    for v in range(_NU_N):
                        g0 = v * h2
                        nc.vector.tensor_tensor(
                            kt_all[:, v * h3:v * h3 + h3],
                            gt_all[:, g0:g0 + h3],
                            gt_all[:, g0 + h3:g0 + h2],
                            op=mybir.AluOpType.mult)
                        k0 = v * h3
                        nc.vector.tensor_tensor(
                            ltw_t[:, v * h4:v * h4 + h4],
                            kt_all[:, k0:k0 + h4],
                            kt_all[:, k0 + h4:k0 + h3],
                            op=mybir.AluOpType.mult)
                    state["pending"] = (ltw_t, 0, _NU_N * h4)
                flush_pending_ln()
            flush_pending_ln()

            # raw accumulators out; final reduction happens on host in f64
            nc.sync.dma_start(out=accp_d.ap(), in_=acc_p[:])
            nc.sync.dma_start(out=accn_d.ap(), in_=acc_n[:])

    nc.compile()
    return nc


def _get_compiled():
    global _compiled
    if _compiled is None:
        _compiled = _build()
    return _compiled


def _prepare(features, anomaly_prob):
    """Host prep: per batch -> (rp, rn) operands + metadata for combine."""
    import ml_dtypes
    feat_all = np.asarray(features, dtype=np.float32)[..., 0]      # (8,64,4000)
    prob_all = np.asarray(anomaly_prob, dtype=np.float32)[:, 0, :, 0]
    BS, C, N = feat_all.shape
    in_maps, metas = [], []
    for b in range(BS):
        feat, prob = feat_all[b], prob_all[b]
        normal = prob < np.float32(0.5)
        nn = int(normal.sum())
        na = N - nn
        # device covers 2048 normal points and 2048 anomaly columns; the
        # host cleans up a small normal overflow. Bail out to numpy if the
        # input distribution is far from the expected ~50/50 split.
        if na > RW or nn - RW > 512:
            return None, None
        norms = np.sqrt(np.sum(feat * feat, axis=0, dtype=np.float32))
        sc = (np.float32(_SQ10) /
              np.maximum(norms, np.float32(1e-12))).astype(np.float32)
        featsc = feat * sc[None, :]
        nd = min(nn, RW)
        fn_all = featsc[:, normal]            # (64, nn) normal points
        fa_all = featsc[:, ~normal]           # (64, na)
        rp = np.zeros((C, RW), np.float32)
        rp[:, :nd] = fn_all[:, :nd]
        rn = np.zeros((C, RW), np.float32)
        rn[:, :na] = fa_all
        rp16 = rp.astype(ml_dtypes.bfloat16)
        rn16 = rn.astype(ml_dtypes.bfloat16)
        # f64 views of the bf16-rounded operands (same values the PE sees)
        rp64 = rp16.astype(np.float64)
        rn64 = rn16.astype(np.float64)
        ov64 = fn_all[:, nd:nn].astype(ml_dtypes.bfloat16).astype(np.float64)

        # host-side diagonal-block sums (f64): D_full = sum over same-block
        # normal-normal pairs (incl. m=n), S2 = sum over m=n only.
        D_full = 0.0
        S2 = 0.0
        for j in range(NBLK):
            r0, r1 = 128 * j, min(128 * (j + 1), nd)
            if r0 >= r1:
                break
            blk = rp64[:, r0:r1]
            sblk = blk.T @ blk
            e = np.exp(sblk)
            D_full += float(e.sum())
            S2 += float(np.trace(e))

        # overflow normals (beyond RW): pos pairs vs all normals + selves,
        # neg pairs vs all anomalies, in f64.
        pos_extra = 0.0
        neg_extra = 0.0
        if nn > nd:
            dev64 = rp64[:, :nd]
            s_cross = ov64.T @ dev64              # (novf, nd)
            pos_extra += 2.0 * float(np.exp(s_cross).sum())
            s_oo = ov64.T @ ov64
            e_oo = np.exp(s_oo)
            pos_extra += float(e_oo.sum()) - float(np.trace(e_oo))
            s_on = ov64.T @ rn64[:, :na]
            sig = 1.0 / (1.0 + np.exp(-s_on))
            neg_extra += float(-np.log(1.0 - sig + EPS).sum())

        metas.append((nn, na, nd, D_full, S2, pos_extra, neg_extra))
        in_maps.append({"rp": rp16, "rn": rn16})
    return in_maps, metas


def _combine(results, metas):
    LN2 = float(np.log(np.float32(2.0)))
    per_batch, n_valid = [], 0
    for r, (nn, na, nd, D_full, S2, pos_extra, neg_extra) in zip(results, metas):
        TC = float(np.asarray(r["accp"], dtype=np.float64).sum())
        TN = float(np.asarray(r["accn"], dtype=np.float64).sum())
        # pos: U-stream block j covers rows [128j,128j+128) x cols
        # [128(j+1), RW). Real (non-padded) entries need row < nd, col < nd.
        fake_c = 0
        for j in range(NBLK - 1):
            rows = min(max(nd - 128 * j, 0), 128)
            cols = max(nd - 128 * (j + 1), 0)
            fake_c += 128 * (RW - 128 * (j + 1)) - rows * cols
        TU_real = TC - float(fake_c)          # exp(0) = 1 exactly
        pos_sum = 2.0 * TU_real + (D_full - S2) + pos_extra
        pos_loss = -np.log(pos_sum / max(nn * (nn - 1), 1) + EPS)
        # neg: stream is RW rows x RW cols; real entries (row < nd, col < na)
        # contribute softplus(s) - 2*ln2 each (the /4 factor scaling),
        # masked entries exactly -ln2 each.
        neg_sum = TN + LN2 * (RW * RW + nd * na) + neg_extra
        neg_loss = neg_sum / max(nn * na, 1)
        if nn >= 10 and na >= 5:
            n_valid += 1
            per_batch.append(pos_loss + neg_loss)
    total = np.sum(per_batch) / max(n_valid, 1) if per_batch else 0.0
    return np.asarray(total, dtype=np.float32)


def _numpy_fallback(features, anomaly_prob):
    feat_all = np.asarray(features, dtype=np.float32)[..., 0]
    prob_all = np.asarray(anomaly_prob, dtype=np.float32)[:, 0, :, 0]
    BS, C, N = feat_all.shape
    per_batch, n_valid = [], 0
    for b in range(BS):
        feat, prob = feat_all[b], prob_all[b]
        normal = prob < 0.5
        nn = int(normal.sum()); na = N - nn
        norms = np.sqrt(np.sum(feat * feat, axis=0, dtype=np.float32))
        fn = feat / np.maximum(norms, 1e-12)[None, :]
        s = (fn.T @ fn) / np.float32(0.1)
        nm, am = normal, ~normal
        eye = np.eye(N, dtype=bool)
        pm = nm[:, None] & nm[None, :] & ~eye
        pos_mean = np.where(pm, np.exp(s), 0.0).sum() / max(pm.sum(), 1)
        pos_loss = -np.log(pos_mean + EPS)
        cm = nm[:, None] & am[None, :]
        neg = np.where(cm, -np.log(1.0 - 1.0 / (1.0 + np.exp(-s)) + EPS),
                       0.0).sum() / max(cm.sum(), 1)
        if nn >= 10 and na >= 5:
            n_valid += 1
            per_batch.append(pos_loss + neg)
    total = np.sum(per_batch) / max(n_valid, 1) if per_batch else 0.0
    return np.asarray(total, dtype=np.float32)


def kernel(features, anomaly_prob):
    from concourse.bass_utils import run_bass_kernel_spmd
    in_maps, metas = _prepare(features, anomaly_prob)
    if in_maps is None:
        return _numpy_fallback(features, anomaly_prob)
    nc = _get_compiled()
    res = run_bass_kernel_spmd(nc, in_maps, list(range(N_CORES)))
    return _combine(res.results, metas)


# revision 46
# speedup vs baseline: 1.1521x; 1.1066x over previous
"""Trainium2 Bass kernel for the contrastive loss problem.

Math (per batch element b, one NeuronCore each):
  feat (C=64, N=4000), prob (N,);  normal = prob < 0.5
  featn = l2-normalize(feat, axis=C);  s = (featn.T @ featn) / 0.1
  pos_loss = -log(mean_{m!=n, both normal} exp(s_mn) + 1e-6)
  neg_loss = mean_{m normal, n anomaly} -log(1 - sigmoid(s_mn) + 1e-6)
  result   = sum_b valid_b * (pos+neg) / max(#valid, 1)

Strategy: data-parallel over batch (8 batches -> 8 cores). Host sorts points
normal-first, scales by sqrt(10) (so the Gram matrix is directly s), and
builds two zero-padded (64, 2048) bf16 operands:
  rp = first min(nn, 2048) normalized normal points
  rn = normalized anomaly points (na <= 2048 for the target inputs)
The device computes, per 128-row block j of rp:
  pos: exp-sum of rp_blk.T @ rp[:, 128j:2048]   (block upper triangle incl.
       the diagonal block; ScalarE fused accumulate out of PSUM)
  neg: softplus-sum of rp_blk.T @ rn, via Exp -> DVE product-fold (8 deep,
       bf16) -> Ln+accumulate (softplus(s) = ln(1+e^s); the |error| vs the
       reference's -log(1-sigmoid(s)+eps) is ~eps*(1+e^s), negligible).
Masked (zero-padded) entries contribute exp(0)=1 / factor 2.0 exactly; the
host subtracts them in closed form. The diagonal 128x128 blocks and any
normal points beyond 2048 are handled on the host in f64 (cheap), which is
what lets the device stream stay at 16 blocks.
"""

import numpy as np

RW = 1920          # device region width = 15 blocks of 128 (all-real core;
                   # points beyond 1920 are ragged remainder -> host f64)
NBLK = RW // 128   # 15 row blocks
UNIT = 2048        # PSUM tile allocation width (bank-aligned); only the
                   # first RW columns are used
N_CORES = 8
EPS = 1e-6
_SQ10 = float(np.sqrt(10.0))


def _unit_segs(block_col_ranges, unitw):
    """Pack a concatenated (block, colrange) matmul stream into units of
    unitw columns, cutting segments at tile-local 512 boundaries (PSUM bank
    safety; tiles are allocated bank-aligned). Returns list of units, each
    a list of (block, c0, c1, tile_off)."""
    units, cur, off = [], [], 0
    for j, cs, ce in block_col_ranges:
        c = cs
        while c < ce:
            take = min(512 - (off % 512), ce - c, unitw - off)
            cur.append((j, c, c + take, off))
            off += take
            c += take
            if off == unitw:
                units.append(cur)
                cur, off = [], 0
    if cur:
        units.append(cur)
    return units


# pos U-stream: per block j, cols [128(j+1), 1920) — strict upper blocks
# only; the diagonal 128x128 blocks are recomputed on the host (D_full).
_POS_UNITS = _unit_segs(
    [(j, 128 * (j + 1), RW) for j in range(NBLK - 1)], RW)  # 7 full units
_NU_P = len(_POS_UNITS)
_NU_N = NBLK                            # 15 neg units of exactly 1920
_N_GRP = 1                              # one Ln group (15 x 120 = 1800)

_compiled = None


def _build():
    import concourse.bass as bass
    import concourse.mybir as mybir
    import concourse.tile as tile
    from concourse import bacc
    from concourse.hw_specs import get_activation_tables

    # Exp and Ln both live in the 'natural_log_exp_and_others' table set, but
    # the default placement resolves them to different sets, causing a ~1.3us
    # ACT table reload on every Exp<->Ln alternation. Steer the placement to
    # the shared set by hiding Exp/Ln from every other set. Set ORDER must be
    # preserved: act_func_set_id is the index into act_info.json's sets, and
    # NRT loads table content by that index.
    def _tables_pref(arch):
        t = get_activation_tables(arch)
        pref = "natural_log_exp_and_others"
        AFt = mybir.ActivationFunctionType
        return {k: (v if k == pref else v - {AFt.Exp, AFt.Ln})
                for k, v in t.items()}

    bacc.get_activation_tables = _tables_pref

    f32 = mybir.dt.float32
    bf16 = mybir.dt.bfloat16
    AF = mybir.ActivationFunctionType

    nc = bacc.Bacc("TRN2", target_bir_lowering=False, debug=False,
                   num_devices=N_CORES)
    rp_d = nc.dram_tensor("rp", [64, RW], bf16, kind="ExternalInput")
    rn_d = nc.dram_tensor("rn", [64, RW], bf16, kind="ExternalInput")
    accp_d = nc.dram_tensor("accp", [128, _NU_P], f32, kind="ExternalOutput")
    accn_d = nc.dram_tensor("accn", [128, _N_GRP], f32, kind="ExternalOutput")

    with tile.TileContext(nc) as tc:
        with (
            tc.tile_pool(name="sb", bufs=1) as sb,
            tc.tile_pool(name="scratch", bufs=2) as scratch_pool,
            tc.tile_pool(name="psum", bufs=2, space=bass.MemorySpace.PSUM) as pp,
        ):
            rp_sb = sb.tile([64, RW], bf16, tag="rp")
            rn_sb = sb.tile([64, RW], bf16, tag="rn")
            # chunked loads on two queues: the first neg unit's matmul
            # segments only need rn[:, c:c+512] and rp[:, 0:128], so the PE
            # can start ~1us earlier than with monolithic loads.
            for c in range(0, RW, 512):
                ce = min(c + 512, RW)
                nc.sync.dma_start(out=rn_sb[:, c:ce],
                                  in_=rn_d.ap()[:, c:ce])
            nc.gpsimd.dma_start(out=rp_sb[:, 0:128], in_=rp_d.ap()[:, 0:128])
            nc.gpsimd.dma_start(out=rp_sb[:, 128:RW],
                                in_=rp_d.ap()[:, 128:RW])

            acc_p = sb.tile([128, _NU_P], f32, tag="accp")
            acc_n = sb.tile([128, _N_GRP], f32, tag="accn")
            # per-unit persistent fold buffers: the last two fold levels are
            # deferred into the pos phase (DVE is idle there), so each neg
            # unit keeps its gt slice alive instead of cycling a ring
            gt_all = sb.tile([128, _NU_N * (RW // 4)], bf16, tag="gt_all")
            kt_all = sb.tile([128, _NU_N * (RW // 8)], bf16, tag="kt_all")
            ltw_t = sb.tile([128, _NU_N * (RW // 16)], bf16, tag="ltw")

            def emit_matmuls(ptile, segs, rhs_sb):
                for (j, c0, c1, off) in segs:
                    nc.tensor.matmul(
                        ptile[:, off:off + (c1 - c0)],
                        rp_sb[:, j * 128:(j + 1) * 128],
                        rhs_sb[:, c0:c1],
                        start=True, stop=True,
                    )

            # The fold tail (kt/lt levels) and the single Ln are deferred
            # into the pos phase, where the DVE and ACT have idle slots.
            state = {"pending": None}

            def flush_pending_ln():
                if state["pending"] is not None:
                    ltw, g, w = state["pending"]
                    ld = scratch_pool.tile([128, UNIT], bf16, tag="fold_o",
                                           name="ld")
                    nc.scalar.activation(ld[:, :w], ltw[:, :w], AF.Ln,
                                         accum_out=acc_n[:, g:g + 1])
                    state["pending"] = None

            # neg phase first: sum softplus(s) = sum ln(1+e^s), with groups
            # of 8 (1+e^s) factors folded by the DVE in bf16 so the Ln pass
            # is 8x narrower (max product (1+e^10)^8 ~ 5.7e34 is inside bf16
            # range). ln(prod) decomposes exactly for masked columns because
            # their factor is exactly 2.0 in bf16.
            for u in range(_NU_N):
                ptile = pp.tile([128, UNIT], f32, tag="unit")
                for c in range(0, RW, 512):
                    ce = min(c + 512, RW)
                    nc.tensor.matmul(
                        ptile[:, c:ce],
                        rp_sb[:, u * 128:(u + 1) * 128],
                        rn_sb[:, c:ce],
                        start=True, stop=True,
                    )
                w = RW
                h1, h2, h3, h4 = w // 2, w // 4, w // 8, w // 16
                et = scratch_pool.tile([128, UNIT], bf16, tag="scratch")
                nc.scalar.activation(et[:, :w], ptile[:, :w], AF.Exp)
                flush_pending_ln()
                # factors are (1+e^s)/4 so a 16-deep fold stays inside bf16
                # range (max (1+e^5.9)^16/4^16 ~ e^72); masked factors become
                # exactly 0.5 (exponent shift, no rounding).
                ft = scratch_pool.tile([128, UNIT], bf16, tag="fold_f")
                nc.vector.tensor_scalar(
                    out=ft[:, :w], in0=et[:, :w], scalar1=0.25, scalar2=0.25,
                    op0=mybir.AluOpType.mult, op1=mybir.AluOpType.add)
                ht = scratch_pool.tile([128, UNIT // 2], bf16, tag="fold_h")
                nc.vector.tensor_tensor(
                    ht[:, :h1], ft[:, :h1], ft[:, h1:w],
                    op=mybir.AluOpType.mult)
                nc.vector.tensor_tensor(
                    gt_all[:, u * h2:u * h2 + h2], ht[:, :h2], ht[:, h2:h1],
                    op=mybir.AluOpType.mult)

            # pos phase (exp-sum, fused accumulate straight out of PSUM);
            # the neg fold tail + Ln are emitted inside it so they overlap
            # pos ACT/PE work on the otherwise-idle DVE
            for u in range(_NU_P):
                ptile = pp.tile([128, UNIT], f32, tag="unit")
                emit_matmuls(ptile, _POS_UNITS[u], rp_sb)
                st = scratch_pool.tile([128, UNIT], bf16, tag="scratch")
                nc.scalar.activation(st[:, :RW], ptile[:, :RW], AF.Exp,
                                     accum_out=acc_p[:, u:u + 1])
                if u == 0:
                    h2, h3, h4 = RW // 4, RW // 8, RW // 16
                    for v in range(_NU_N):
                        g0 = v * h2
                        nc.vector.tensor_tensor(
                            kt_all[:, v * h3:v * h3 + h3],
                            gt_all[:, g0:g0 + h3],
                            gt_all[:, g0 + h3:g0 + h2],
                            op=mybir.AluOpType.mult)
                        k0 = v * h3
                        nc.vector.tensor_tensor(
                            ltw_t[:, v * h4:v * h4 + h4],
                            kt_all[:, k0:k0 + h4],
                            kt_all[:, k0 + h4:k0 + h3],
                            op=mybir.AluOpType.mult)
                    state["pending"] = (ltw_t, 0, _NU_N * h4)
                flush_pending_ln()
            flush_pending_ln()

            # raw accumulators out; final reduction happens on host in f64
            nc.sync.dma_start(out=accp_d.ap(), in_=acc_p[:])
            nc.sync.dma_start(out=accn_d.ap(), in_=acc_n[:])

    nc.compile()
    return nc


def _get_compiled():
    global _compiled
    if _compiled is None:
        _compiled = _build()
    return _compiled


def _prepare(features, anomaly_prob):
    """Host prep: per batch -> (rp, rn) operands + metadata for combine."""
    import ml_dtypes
    feat_all = np.asarray(features, dtype=np.float32)[..., 0]      # (8,64,4000)
    prob_all = np.asarray(anomaly_prob, dtype=np.float32)[:, 0, :, 0]
    BS, C, N = feat_all.shape
    in_maps, metas = [], []
    for b in range(BS):
        feat, prob = feat_all[b], prob_all[b]
        normal = prob < np.float32(0.5)
        nn = int(normal.sum())
        na = N - nn
        # device covers the dense 1920x1920 core; the host cleans up the
        # ragged remainder. Bail out to numpy if the input distribution is
        # far from the expected ~50/50 split.
        if na - RW > 512 or nn - RW > 512:
            return None, None
        norms = np.sqrt(np.sum(feat * feat, axis=0, dtype=np.float32))
        sc = (np.float32(_SQ10) /
              np.maximum(norms, np.float32(1e-12))).astype(np.float32)
        featsc = feat * sc[None, :]
        nd = min(nn, RW)
        na_dev = min(na, RW)
        fn_all = featsc[:, normal]            # (64, nn) normal points
        fa_all = featsc[:, ~normal]           # (64, na)
        rp = np.zeros((C, RW), np.float32)
        rp[:, :nd] = fn_all[:, :nd]
        rn = np.zeros((C, RW), np.float32)
        rn[:, :na_dev] = fa_all[:, :na_dev]
        rp16 = rp.astype(ml_dtypes.bfloat16)
        rn16 = rn.astype(ml_dtypes.bfloat16)
        # f64 views of the bf16-rounded operands (same values the PE sees)
        rp64 = rp16.astype(np.float64)
        rnf64 = fa_all.astype(ml_dtypes.bfloat16).astype(np.float64)  # all na
        ov64 = fn_all[:, nd:nn].astype(ml_dtypes.bfloat16).astype(np.float64)

        # host-side diagonal-block sums (f64): D_full = sum over same-block
        # normal-normal pairs (incl. m=n), S2 = sum over m=n only.
        D_full = 0.0
        S2 = 0.0
        for j in range(NBLK):
            r0, r1 = 128 * j, min(128 * (j + 1), nd)
            if r0 >= r1:
                break
            blk = rp64[:, r0:r1]
            sblk = blk.T @ blk
            e = np.exp(sblk)
            D_full += float(e.sum())
            S2 += float(np.trace(e))

        # overflow normals (beyond RW): pos pairs vs all normals + selves,
        # neg pairs vs all anomalies, in f64.
        pos_extra = 0.0
        neg_extra = 0.0
        if nn > nd:
            dev64 = rp64[:, :nd]
            s_cross = ov64.T @ dev64              # (novf, nd)
            pos_extra += 2.0 * float(np.exp(s_cross).sum())
            s_oo = ov64.T @ ov64
            e_oo = np.exp(s_oo)
            pos_extra += float(e_oo.sum()) - float(np.trace(e_oo))
            s_on = ov64.T @ rnf64
            sig = 1.0 / (1.0 + np.exp(-s_on))
            neg_extra += float(-np.log(1.0 - sig + EPS).sum())
        if na > na_dev:
            # device normals x remainder anomaly columns
            s_rem = rp64[:, :nd].T @ rnf64[:, na_dev:na]
            sig = 1.0 / (1.0 + np.exp(-s_rem))
            neg_extra += float(-np.log(1.0 - sig + EPS).sum())

        metas.append((nn, na, nd, na_dev, D_full, S2, pos_extra, neg_extra))
        in_maps.append({"rp": rp16, "rn": rn16})
    return in_maps, metas


def _combine(results, metas):
    LN2 = float(np.log(np.float32(2.0)))
    per_batch, n_valid = [], 0
    for r, (nn, na, nd, na_dev, D_full, S2, pos_extra,
            neg_extra) in zip(results, metas):
        TC = float(np.asarray(r["accp"], dtype=np.float64).sum())
        TN = float(np.asarray(r["accn"], dtype=np.float64).sum())
        # pos: U-stream block j covers rows [128j,128j+128) x cols
        # [128(j+1), RW). Real (non-padded) entries need row < nd, col < nd.
        fake_c = 0
        for j in range(NBLK - 1):
            rows = min(max(nd - 128 * j, 0), 128)
            cols = max(nd - 128 * (j + 1), 0)
            fake_c += 128 * (RW - 128 * (j + 1)) - rows * cols
        TU_real = TC - float(fake_c)          # exp(0) = 1 exactly
        pos_sum = 2.0 * TU_real + (D_full - S2) + pos_extra
        pos_loss = -np.log(pos_sum / max(nn * (nn - 1), 1) + EPS)
        # neg: stream is RW rows x RW cols; real entries (row < nd,
        # col < na_dev) contribute softplus(s) - 2*ln2 each (the /4 factor
        # scaling), masked entries exactly -ln2 each.
        neg_sum = TN + LN2 * (RW * RW + nd * na_dev) + neg_extra
        neg_loss = neg_sum / max(nn * na, 1)
        if nn >= 10 and na >= 5:
            n_valid += 1
            per_batch.append(pos_loss + neg_loss)
    total = np.sum(per_batch) / max(n_valid, 1) if per_batch else 0.0
    return np.asarray(total, dtype=np.float32)


def _numpy_fallback(features, anomaly_prob):
    feat_all = np.asarray(features, dtype=np.float32)[..., 0]
    prob_all = np.asarray(anomaly_prob, dtype=np.float32)[:, 0, :, 0]
    BS, C, N = feat_all.shape
    per_batch, n_valid = [], 0
    for b in range(BS):
        feat, prob = feat_all[b], prob_all[b]
        normal = prob < 0.5
        nn = int(normal.sum()); na = N - nn
        norms = np.sqrt(np.sum(feat * feat, axis=0, dtype=np.float32))
        fn = feat / np.maximum(norms, 1e-12)[None, :]
        s = (fn.T @ fn) / np.float32(0.1)
        nm, am = normal, ~normal
        eye = np.eye(N, dtype=bool)
        pm = nm[:, None] & nm[None, :] & ~eye
        pos_mean = np.where(pm, np.exp(s), 0.0).sum() / max(pm.sum(), 1)
        pos_loss = -np.log(pos_mean + EPS)
        cm = nm[:, None] & am[None, :]
        neg = np.where(cm, -np.log(1.0 - 1.0 / (1.0 + np.exp(-s)) + EPS),
                       0.0).sum() / max(cm.sum(), 1)
        if nn >= 10 and na >= 5:
            n_valid += 1
            per_batch.append(pos_loss + neg)
    total = np.sum(per_batch) / max(n_valid, 1) if per_batch else 0.0
    return np.asarray(total, dtype=np.float32)


def kernel(features, anomaly_prob):
    from concourse.bass_utils import run_bass_kernel_spmd
    in_maps, metas = _prepare(features, anomaly_prob)
    if in_maps is None:
        return _numpy_fallback(features, anomaly_prob)
    nc = _get_compiled()
    res = run_bass_kernel_spmd(nc, in_maps, list(range(N_CORES)))
    return _combine(res.results, metas)
